# revision 19
# baseline (speedup 1.0000x reference)
"""Trainium2 Bass kernel for nn_LowRankOrthogonalMixer.

Math: the reference builds per-batch skew matrices G = gate*(A - A^T) with
A = (left*coeff) @ right^T (rank<=16 each term), combines them into
Omega = 0.5*(G+L) + comm/12*(LG-GL), applies the Cayley transform
T = (I-0.5*Omega)^{-1}(I+0.5*Omega), and mixes: out = x @ T.

Key structure: Omega = P @ S @ Q^T with P,Q in [F,64] (rank<=64), so with
W = 0.5*P*S (0.5*Omega = W Q^T) and C = I64 - Q^T W, Woodbury gives
    T = I + 2 W C^{-1} Q^T
exactly, and
    out = x + (x @ W) @ (2 C^{-1}) @ Q^T.
C (64x64) is inverted on-device by Newton-Schulz iteration (C has
eigenvalues 1 +/- 0.5i*mu, well conditioned).

Sharding: data-parallel over batch B=8 -> one batch item per NeuronCore.
"""

import numpy as np

import concourse.bass as bass
import concourse.bacc as bacc
import concourse.tile as tile
from concourse import mybir
from concourse.bass_utils import run_bass_kernel_spmd

B, N, F, R = 8, 4096, 512, 16
NTILES = N // 128
ALPHA = 1.0 / 9.0  # Newton-Schulz init scale: V0 = ALPHA * C^T (safe: sigma_max(C)^2 < 2/ALPHA;
# measured sigma(C) in [0.55, 2.9] across batches -> alpha*sigma_max^2 = 0.93, margin 2.1x)
NS_ITERS = 10

# packed setup tensor column layout
_C_SMALLS = 0        # [128, 512]: rows 0:64 Q^T sources, 64:128 P^T sources
_C_IDENT = 512       # [128, 128] identity
_C_E0 = 640          # [64, 64] +-1/24 commutator mask (rows 64:128 zero)
_C_BASE = 704        # qp_base column
_C_GATE = 705        # qp_gate column
_C_SIGN = 706        # qp_sign column
_C_CVEC = 707        # comm_scale broadcast column (rows 0:64)
SETUP_COLS = 708

_CACHE = {}


def build_bass():
    # Bacc (not plain Bass): its compile() runs move_matmul_waits_to_ldweights
    # + generate_event_semaphores, required because TRN2 instructions support
    # at most one semaphore wait each.
    nc = bacc.Bacc(trn_type="TRN2", target_bir_lowering=False)
    dt = mybir.dt.float32
    bf16 = mybir.dt.bfloat16

    x_d = nc.dram_tensor("x", [N, F], dt, kind="ExternalInput")
    setup_d = nc.dram_tensor("setup", [128, SETUP_COLS], dt, kind="ExternalInput")
    out_d = nc.dram_tensor("out", [N, F], dt, kind="ExternalOutput")

    with tile.TileContext(nc) as tc:
        with (
            tc.tile_pool(name="const", bufs=1) as const,
            tc.tile_pool(name="small", bufs=2) as small,
            # deep buffering: phase 1's front (DMA/transpose/mm1) must be able
            # to run ~NTILES tiles ahead while phase 0's Newton-Schulz chain
            # (which gates mm2 via ztm) completes
            tc.tile_pool(name="xs", bufs=NTILES + 2) as xs,
            tc.tile_pool(name="xts", bufs=4) as xts,
            tc.tile_pool(name="us", bufs=NTILES // 2 + 2) as us,
            tc.tile_pool(name="outs", bufs=8) as outs,
            tc.tile_pool(name="ps_sm", bufs=2, space="PSUM") as ps_sm,
            tc.tile_pool(name="ps_str", bufs=2, space="PSUM") as ps_str,
            tc.tile_pool(name="ps_u", bufs=2, space="PSUM") as ps_u_pool,
            tc.tile_pool(name="ps_o", bufs=2, space="PSUM") as ps_o_pool,
        ):
            ps_once = ps_sm
            # ---- load packed setup (1 DMA) + base-partition-0 copy of the P^T half ----
            setup = const.tile([128, SETUP_COLS], dt)
            nc.sync.dma_start(setup, setup_d[:, :])
            setup_p = const.tile([64, SETUP_COLS], dt)
            nc.sync.dma_start(setup_p, setup_d[64:128, :])

            smalls = setup[:, _C_SMALLS:_C_SMALLS + 512]
            ident = setup[:, _C_IDENT:_C_IDENT + 128]
            i64 = setup[0:64, _C_IDENT:_C_IDENT + 64]
            e0 = setup[0:64, _C_E0:_C_E0 + 64]
            base_v = setup[:, _C_BASE:_C_BASE + 1]
            gate_v = setup[:, _C_GATE:_C_GATE + 1]
            sign_v = setup[:, _C_SIGN:_C_SIGN + 1]
            cv = setup[0:64, _C_CVEC:_C_CVEC + 1]

            # ---- phase 0: build W (natural), Q^T, C, C^{-1}, ZT = 2 C^{-1} Q^T ----
            # qp rows 0:64 = Q^T, rows 64:128 = P^T
            scale = small.tile([128, 1], dt, tag="scale")
            nc.vector.tensor_mul(scale, base_v, gate_v)
            scale2 = small.tile([128, 1], dt, tag="scale2")
            nc.vector.tensor_mul(scale2, scale, sign_v)
            qp = const.tile([128, F], dt)
            nc.vector.tensor_scalar_mul(qp, in0=smalls, scalar1=scale2)
            qt_ap = qp[0:64, :]
            # P^T at base partition 0 for the W^T matmul
            scale_p = small.tile([64, 1], dt, tag="scale_p")
            nc.vector.tensor_mul(
                scale_p,
                setup_p[:, _C_BASE:_C_BASE + 1],
                setup_p[:, _C_GATE:_C_GATE + 1],
            )
            scale_p2 = small.tile([64, 1], dt, tag="scale_p2")
            nc.vector.tensor_mul(scale_p2, scale_p, setup_p[:, _C_SIGN:_C_SIGN + 1])
            pt0 = const.tile([64, F], dt)
            nc.vector.tensor_scalar_mul(
                pt0, in0=setup_p[:, _C_SMALLS:_C_SMALLS + 512], scalar1=scale_p2
            )

            # naturals: qpn block c (cols 128c..128c+128) = (qp[:,128c:128c+128])^T
            ps_qpn = ps_once.tile([128, 512], dt, tag="ns_ps")
            for c in range(4):
                nc.tensor.transpose(
                    ps_qpn[:, 128 * c : 128 * (c + 1)],
                    qp[:, 128 * c : 128 * (c + 1)],
                    ident,
                )
            qpn = const.tile([128, 512], dt)
            nc.scalar.copy(qpn, ps_qpn)

            # G1 = Q^T P, G1T = P^T Q   (accumulate over 4 F-chunks)
            ps_g1 = ps_sm.tile([64, 128], dt, tag="ns_ps")
            for c in range(4):
                qch = qpn[:, 128 * c : 128 * c + 64]
                pch = qpn[:, 128 * c + 64 : 128 * (c + 1)]
                nc.tensor.matmul(ps_g1[:, 0:64], qch, pch, start=(c == 0), stop=(c == 3))
            g1 = small.tile([64, 64], dt, tag="g1")
            nc.scalar.copy(g1, ps_g1[:, 0:64])
            ps_g1t = ps_sm.tile([64, 128], dt, tag="ns_ps")
            for c in range(4):
                qch = qpn[:, 128 * c : 128 * c + 64]
                pch = qpn[:, 128 * c + 64 : 128 * (c + 1)]
                nc.tensor.matmul(ps_g1t[:, 0:64], pch, qch, start=(c == 0), stop=(c == 3))
            g1t = small.tile([64, 64], dt, tag="g1t")
            nc.scalar.copy(g1t, ps_g1t[:, 0:64])

            # S_half = 0.25*I + comm * (e0 ⊙ G1)   (e0 carries the ±1/24 pattern)
            e0c = small.tile([64, 64], dt, tag="e0c")
            nc.vector.tensor_scalar_mul(e0c, in0=e0, scalar1=cv)
            s_half = small.tile([64, 64], dt, tag="s_half")
            nc.vector.tensor_mul(s_half, e0c, g1)
            i4 = small.tile([64, 64], dt, tag="i4")
            nc.scalar.mul(i4, i64, 0.25)
            nc.vector.tensor_add(s_half, s_half, i4)

            # C = I - G1 @ S_half ;  C^T = I - S_half^T @ G1^T
            ps_cr = ps_sm.tile([64, 128], dt, tag="ns_ps")
            nc.tensor.matmul(ps_cr[:, 0:64], g1t, s_half, start=True, stop=True)
            cmat = small.tile([64, 64], dt, tag="cmat")
            nc.vector.tensor_sub(cmat, i64, ps_cr[:, 0:64])
            ps_ctr = ps_sm.tile([64, 128], dt, tag="ns_ps")
            nc.tensor.matmul(ps_ctr[:, 0:64], s_half, g1t, start=True, stop=True)
            ctm = small.tile([64, 64], dt, tag="ctm")
            nc.vector.tensor_sub(ctm, i64, ps_ctr[:, 0:64])

            # W^T = S_half^T @ P^T  [64, F]; then W natural in 4 chunks [128, 64]
            ps_wt = ps_once.tile([128, 512], dt, tag="ns_ps")
            nc.tensor.matmul(ps_wt[0:64, :], s_half, pt0, start=True, stop=True)
            wtm = const.tile([64, 512], dt)
            nc.scalar.copy(wtm, ps_wt[0:64, :])
            ps_w = ps_once.tile([128, 512], dt, tag="ns_ps")
            for c in range(4):
                nc.tensor.transpose(
                    ps_w[:, 64 * c : 64 * (c + 1)],
                    wtm[:, 128 * c : 128 * (c + 1)],
                    i64,
                )
            wm = const.tile([128, 256], bf16)
            nc.scalar.copy(wm, ps_w[:, 0:256])

            # Newton-Schulz for V = C^{-1} (maintains V and V^T)
            i2 = small.tile([64, 64], dt, tag="i2")
            nc.scalar.mul(i2, i64, 2.0)
            v = small.tile([64, 64], dt, tag="v")
            nc.scalar.mul(v, ctm, ALPHA)
            vt = small.tile([64, 64], dt, tag="vt")
            nc.scalar.mul(vt, cmat, ALPHA)
            for _ in range(NS_ITERS):
                ps_t1 = ps_sm.tile([64, 128], dt, tag="ns_ps")
                nc.tensor.matmul(ps_t1[:, 0:64], ctm, v, start=True, stop=True)  # C V
                t2 = small.tile([64, 64], dt, tag="t2")
                nc.vector.tensor_sub(t2, i2, ps_t1[:, 0:64])  # 2I - CV
                ps_v = ps_sm.tile([64, 128], dt, tag="ns_ps")
                nc.tensor.matmul(ps_v[:, 0:64], vt, t2, start=True, stop=True)  # V t2
                ps_vt = ps_sm.tile([64, 128], dt, tag="ns_ps")
                nc.tensor.matmul(ps_vt[:, 0:64], t2, vt, start=True, stop=True)  # t2^T V^T
                v = small.tile([64, 64], dt, tag="v")
                nc.scalar.copy(v, ps_v[:, 0:64])
                vt_new = small.tile([64, 64], dt, tag="vt")
                nc.scalar.copy(vt_new, ps_vt[:, 0:64])
                vt = vt_new

            # ZT = 2 * V @ Q^T  [64, F]
            ps_zt = ps_once.tile([128, 512], dt, tag="ns_ps")
            nc.tensor.matmul(ps_zt[0:64, :], vt, qt_ap, start=True, stop=True)
            ztm = const.tile([64, 512], bf16)
            nc.scalar.mul(ztm, ps_zt[0:64, :], 2.0)

            # bf16 identity for the transpose-by-matmul trick
            identb = const.tile([128, 128], bf16)
            nc.scalar.copy(identb, ident)

            # ---- phase 1: stream x tiles in groups of 4 ----
            # The correction term (~17% of output magnitude) runs in bf16 on
            # PE; the residual add keeps x in full fp32. Transposes are plain
            # bf16 matmuls against the identity (these pipeline back-to-back,
            # unlike transpose-mode which pays the SBUF-access latency per op).
            GT = 4
            x_t = x_d[:, :].rearrange("(t p) f -> t p f", p=128)
            o_t = out_d[:, :].rearrange("(t p) f -> t p f", p=128)
            for g in range(NTILES // GT):
                xi_grp = []
                # xt4 layout [128, (c t n)]: chunk c of all GT tiles adjacent so
                # mm1's rhs for chunk c is the contiguous slice [:, 512c:512c+512]
                xt4 = xts.tile([128, GT * 512], bf16, tag="xt4")
                xt4_v = xt4[:, :].rearrange("p (c t n) -> p c t n", c=4, t=GT)
                for t in range(GT):
                    xi = xs.tile([128, 512], dt, tag="xi")
                    nc.sync.dma_start(xi, x_t[GT * g + t])
                    xi_grp.append(xi)
                    xb = xts.tile([128, 512], bf16, tag="xb")
                    nc.scalar.copy(xb, xi)
                    ps_xt = ps_str.tile([128, 512], dt, tag="ps_xt")
                    for c in range(4):
                        nc.tensor.matmul(
                            ps_xt[:, 128 * c : 128 * (c + 1)],
                            xb[:, 128 * c : 128 * (c + 1)],
                            identb,
                            start=True,
                            stop=True,
                        )
                    nc.vector.tensor_copy(
                        xt4_v[:, :, t, :],
                        ps_xt[:, :].rearrange("p (c n) -> p c n", c=4),
                    )
                ps_u4 = ps_u_pool.tile([64, 512], dt, tag="ps_u2")
                for c in range(4):
                    nc.tensor.matmul(
                        ps_u4,
                        wm[:, 64 * c : 64 * (c + 1)],
                        xt4[:, 512 * c : 512 * (c + 1)],
                        start=(c == 0),
                        stop=(c == 3),
                    )
                u4 = us.tile([64, 512], bf16, tag="u2")
                nc.scalar.copy(u4, ps_u4)
                for t in range(GT):
                    ps_o = ps_o_pool.tile([128, 512], dt, tag="ps_o")
                    nc.tensor.matmul(
                        ps_o,
                        u4[:, 128 * t : 128 * (t + 1)],
                        ztm,
                        start=True,
                        stop=True,
                    )
                    ob = outs.tile([128, 512], dt, tag="ob")
                    nc.vector.tensor_add(ob, xi_grp[t], ps_o)
                    nc.sync.dma_start(o_t[GT * g + t], ob)

    return nc


def make_setup(coeff_b, gate_b, coeff_l_b, gate_l_b, comm_b,
               left, right, left_local, right_local):
    """Pack all small inputs for one batch item into one [128, 708] tensor.
    Pure marshalling: transposes/replication of raw inputs plus constants."""
    f32 = np.float32
    s = np.zeros((128, SETUP_COLS), f32)
    s[:, 0:512] = np.concatenate(
        [right.T, left.T, right_local.T, left_local.T,
         left.T, right.T, left_local.T, right_local.T], axis=0
    )
    s[:, _C_IDENT:_C_IDENT + 128] = np.eye(128, dtype=f32)
    s[0:32, _C_E0 + 32:_C_E0 + 64] = -1.0 / 24.0
    s[32:64, _C_E0:_C_E0 + 32] = 1.0 / 24.0
    ones16 = np.ones(16, f32)
    s[:, _C_BASE] = np.concatenate(
        [ones16, coeff_b, ones16, coeff_l_b, coeff_b, ones16, coeff_l_b, ones16]
    )
    s[:, _C_GATE] = np.concatenate(
        [np.ones(64, f32), np.full(32, gate_b, f32), np.full(32, gate_l_b, f32)]
    )
    s[:, _C_SIGN] = np.concatenate(
        [np.ones(80, f32), -np.ones(16, f32), np.ones(16, f32), -np.ones(16, f32)]
    )
    s[0:64, _C_CVEC] = comm_b
    return s


def make_in_maps(x, coeff, gate, coeff_local, gate_local, comm_scale,
                 left, right, left_local, right_local):
    in_maps = []
    for b in range(x.shape[0]):
        in_maps.append({
            "x": np.ascontiguousarray(x[b]).astype(np.float32),
            "setup": make_setup(coeff[b], gate[b], coeff_local[b], gate_local[b],
                                comm_scale[b], left, right, left_local, right_local),
        })
    return in_maps


def kernel(x, coeff, gate, coeff_local, gate_local, comm_scale,
           left, right, left_local, right_local, _trace=False):
    if "nc" not in _CACHE:
        nc = build_bass()
        nc.finalize()  # Bacc.finalize: compile passes + freeze
        _CACHE["nc"] = nc
    nc = _CACHE["nc"]
    in_maps = make_in_maps(x, coeff, gate, coeff_local, gate_local, comm_scale,
                           left, right, left_local, right_local)
    res = run_bass_kernel_spmd(nc, in_maps, core_ids=list(range(8)), trace=_trace)
    out = np.stack([r["out"] for r in res.results], axis=0)
    if _trace:
        _CACHE["last_results"] = res
    return out.astype(x.dtype)


# revision 21
# speedup vs baseline: 1.1187x; 1.1187x over previous
"""Trainium2 Bass kernel for nn_LowRankOrthogonalMixer.

Math: the reference builds per-batch skew matrices G = gate*(A - A^T) with
A = (left*coeff) @ right^T (rank<=16 each term), combines them into
Omega = 0.5*(G+L) + comm/12*(LG-GL), applies the Cayley transform
T = (I-0.5*Omega)^{-1}(I+0.5*Omega), and mixes: out = x @ T.

Key structure: Omega = P @ S @ Q^T with P,Q in [F,64] (rank<=64), so with
W = 0.5*P*S (0.5*Omega = W Q^T) and C = I64 - Q^T W, Woodbury gives
    T = I + 2 W C^{-1} Q^T
exactly, and
    out = x + (x @ W) @ (2 C^{-1}) @ Q^T.
C (64x64) is inverted on-device by Newton-Schulz iteration (C has
eigenvalues 1 +/- 0.5i*mu, well conditioned).

Sharding: data-parallel over batch B=8 -> one batch item per NeuronCore.
"""

import numpy as np

import concourse.bass as bass
import concourse.bacc as bacc
import concourse.tile as tile
from concourse import mybir
from concourse.bass_utils import run_bass_kernel_spmd

B, N, F, R = 8, 4096, 512, 16
NTILES = N // 128
ALPHA = 1.0 / 9.0  # Newton-Schulz init scale: V0 = ALPHA * C^T (safe: sigma_max(C)^2 < 2/ALPHA;
# measured sigma(C) in [0.55, 2.9] across batches -> alpha*sigma_max^2 = 0.93, margin 2.1x)
NS_ITERS = 10

# packed setup tensor column layout
_C_SMALLS = 0        # [128, 512]: rows 0:64 Q^T sources, 64:128 P^T sources
_C_IDENT = 512       # [128, 128] identity
_C_E0 = 640          # [64, 64] +-1/24 commutator mask (rows 64:128 zero)
_C_BASE = 704        # qp_base column
_C_GATE = 705        # qp_gate column
_C_SIGN = 706        # qp_sign column
_C_CVEC = 707        # comm_scale broadcast column (rows 0:64)
SETUP_COLS = 708

_CACHE = {}


def build_bass():
    # Bacc (not plain Bass): its compile() runs move_matmul_waits_to_ldweights
    # + generate_event_semaphores, required because TRN2 instructions support
    # at most one semaphore wait each.
    nc = bacc.Bacc(trn_type="TRN2", target_bir_lowering=False)
    dt = mybir.dt.float32
    bf16 = mybir.dt.bfloat16
    f32r = mybir.dt.float32r

    x_d = nc.dram_tensor("x", [N, F], dt, kind="ExternalInput")
    setup_d = nc.dram_tensor("setup", [128, SETUP_COLS], dt, kind="ExternalInput")
    out_d = nc.dram_tensor("out", [N, F], dt, kind="ExternalOutput")

    with tile.TileContext(nc) as tc:
        with (
            tc.tile_pool(name="const", bufs=1) as const,
            tc.tile_pool(name="small", bufs=2) as small,
            # deep buffering: phase 1's front (DMA/transpose/mm1) must be able
            # to run ~NTILES tiles ahead while phase 0's Newton-Schulz chain
            # (which gates mm2 via ztm) completes
            tc.tile_pool(name="xs", bufs=NTILES + 2) as xs,
            tc.tile_pool(name="xts", bufs=4) as xts,
            tc.tile_pool(name="us", bufs=NTILES // 4 + 2) as us,
            tc.tile_pool(name="outs", bufs=8) as outs,
            tc.tile_pool(name="ps_sm", bufs=2, space="PSUM") as ps_sm,
            tc.tile_pool(name="ps_str", bufs=2, space="PSUM") as ps_str,
            tc.tile_pool(name="ps_u", bufs=2, space="PSUM") as ps_u_pool,
            tc.tile_pool(name="ps_o", bufs=2, space="PSUM") as ps_o_pool,
        ):
            ps_once = ps_sm
            # PE warm-up: ~6us of dense dummy matmuls so the HAM clock gate
            # opens (K=8/8, 2.4 GHz) early; otherwise the whole kernel can run
            # at the cold 1.2 GHz PE clock.
            warm_src = const.tile([128, 128], bf16)
            nc.vector.memset(warm_src, 0.0)
            ps_warm = ps_sm.tile([128, 512], dt, tag="ns_ps")
            for _ in range(56):
                nc.tensor.matmul(ps_warm[:, 0:128], warm_src, warm_src,
                                 start=True, stop=True)

            # ---- load packed setup (1 DMA) + base-partition-0 copy of the P^T half ----
            setup = const.tile([128, SETUP_COLS], dt)
            nc.sync.dma_start(setup, setup_d[:, :])
            setup_p = const.tile([64, SETUP_COLS], dt)
            nc.sync.dma_start(setup_p, setup_d[64:128, :])

            smalls = setup[:, _C_SMALLS:_C_SMALLS + 512]
            ident = setup[:, _C_IDENT:_C_IDENT + 128]
            i64 = setup[0:64, _C_IDENT:_C_IDENT + 64]
            e0 = setup[0:64, _C_E0:_C_E0 + 64]
            base_v = setup[:, _C_BASE:_C_BASE + 1]
            gate_v = setup[:, _C_GATE:_C_GATE + 1]
            sign_v = setup[:, _C_SIGN:_C_SIGN + 1]
            cv = setup[0:64, _C_CVEC:_C_CVEC + 1]

            # ---- phase 0: build W (natural), Q^T, C, C^{-1}, ZT = 2 C^{-1} Q^T ----
            # qp rows 0:64 = Q^T, rows 64:128 = P^T
            scale = small.tile([128, 1], dt, tag="scale")
            nc.vector.tensor_mul(scale, base_v, gate_v)
            scale2 = small.tile([128, 1], dt, tag="scale2")
            nc.vector.tensor_mul(scale2, scale, sign_v)
            qp = const.tile([128, F], dt)
            nc.vector.tensor_scalar_mul(qp, in0=smalls, scalar1=scale2)
            qt_ap = qp[0:64, :]
            # P^T at base partition 0 for the W^T matmul
            scale_p = small.tile([64, 1], dt, tag="scale_p")
            nc.vector.tensor_mul(
                scale_p,
                setup_p[:, _C_BASE:_C_BASE + 1],
                setup_p[:, _C_GATE:_C_GATE + 1],
            )
            scale_p2 = small.tile([64, 1], dt, tag="scale_p2")
            nc.vector.tensor_mul(scale_p2, scale_p, setup_p[:, _C_SIGN:_C_SIGN + 1])
            pt0 = const.tile([64, F], dt)
            nc.vector.tensor_scalar_mul(
                pt0, in0=setup_p[:, _C_SMALLS:_C_SMALLS + 512], scalar1=scale_p2
            )

            # naturals: qpn block c (cols 128c..128c+128) = (qp[:,128c:128c+128])^T
            ps_qpn = ps_once.tile([128, 512], dt, tag="ns_ps")
            for c in range(4):
                nc.tensor.transpose(
                    ps_qpn[:, 128 * c : 128 * (c + 1)],
                    qp[:, 128 * c : 128 * (c + 1)],
                    ident,
                )
            qpn = const.tile([128, 512], dt)
            nc.scalar.copy(qpn, ps_qpn)

            # G1 = Q^T P, G1T = P^T Q   (accumulate over 4 F-chunks)
            ps_g1 = ps_sm.tile([64, 128], dt, tag="ns_ps")
            for c in range(4):
                qch = qpn[:, 128 * c : 128 * c + 64]
                pch = qpn[:, 128 * c + 64 : 128 * (c + 1)]
                nc.tensor.matmul(ps_g1[:, 0:64], qch, pch, start=(c == 0), stop=(c == 3))
            g1 = small.tile([64, 64], dt, tag="g1")
            nc.scalar.copy(g1, ps_g1[:, 0:64])
            ps_g1t = ps_sm.tile([64, 128], dt, tag="ns_ps")
            for c in range(4):
                qch = qpn[:, 128 * c : 128 * c + 64]
                pch = qpn[:, 128 * c + 64 : 128 * (c + 1)]
                nc.tensor.matmul(ps_g1t[:, 0:64], pch, qch, start=(c == 0), stop=(c == 3))
            g1t = small.tile([64, 64], dt, tag="g1t")
            nc.scalar.copy(g1t, ps_g1t[:, 0:64])

            # S_half = 0.25*I + comm * (e0 ⊙ G1)   (e0 carries the ±1/24 pattern)
            e0c = small.tile([64, 64], dt, tag="e0c")
            nc.vector.tensor_scalar_mul(e0c, in0=e0, scalar1=cv)
            s_half = small.tile([64, 64], dt, tag="s_half")
            nc.vector.tensor_mul(s_half, e0c, g1)
            i4 = small.tile([64, 64], dt, tag="i4")
            nc.scalar.mul(i4, i64, 0.25)
            nc.vector.tensor_add(s_half, s_half, i4)

            # C = I - G1 @ S_half ;  C^T = I - S_half^T @ G1^T
            ps_cr = ps_sm.tile([64, 128], dt, tag="ns_ps")
            nc.tensor.matmul(ps_cr[:, 0:64], g1t, s_half, start=True, stop=True)
            cmat = small.tile([64, 64], dt, tag="cmat")
            nc.vector.tensor_sub(cmat, i64, ps_cr[:, 0:64])
            ps_ctr = ps_sm.tile([64, 128], dt, tag="ns_ps")
            nc.tensor.matmul(ps_ctr[:, 0:64], s_half, g1t, start=True, stop=True)
            ctm = small.tile([64, 64], dt, tag="ctm")
            nc.vector.tensor_sub(ctm, i64, ps_ctr[:, 0:64])

            # W^T = S_half^T @ P^T  [64, F]; then W natural in 4 chunks [128, 64]
            ps_wt = ps_once.tile([128, 512], dt, tag="ns_ps")
            nc.tensor.matmul(ps_wt[0:64, :], s_half, pt0, start=True, stop=True)
            wtm = const.tile([64, 512], dt)
            nc.scalar.copy(wtm, ps_wt[0:64, :])
            ps_w = ps_once.tile([128, 512], dt, tag="ns_ps")
            for c in range(4):
                nc.tensor.transpose(
                    ps_w[:, 64 * c : 64 * (c + 1)],
                    wtm[:, 128 * c : 128 * (c + 1)],
                    i64,
                )
            wm = const.tile([128, 256], f32r)
            nc.scalar.copy(wm, ps_w[:, 0:256])

            # Newton-Schulz for V = C^{-1} (maintains V and V^T)
            i2 = small.tile([64, 64], dt, tag="i2")
            nc.scalar.mul(i2, i64, 2.0)
            v = small.tile([64, 64], dt, tag="v")
            nc.scalar.mul(v, ctm, ALPHA)
            vt = small.tile([64, 64], dt, tag="vt")
            nc.scalar.mul(vt, cmat, ALPHA)
            for _ in range(NS_ITERS):
                ps_t1 = ps_sm.tile([64, 128], dt, tag="ns_ps")
                nc.tensor.matmul(ps_t1[:, 0:64], ctm, v, start=True, stop=True)  # C V
                t2 = small.tile([64, 64], dt, tag="t2")
                nc.vector.tensor_sub(t2, i2, ps_t1[:, 0:64])  # 2I - CV
                ps_v = ps_sm.tile([64, 128], dt, tag="ns_ps")
                nc.tensor.matmul(ps_v[:, 0:64], vt, t2, start=True, stop=True)  # V t2
                ps_vt = ps_sm.tile([64, 128], dt, tag="ns_ps")
                nc.tensor.matmul(ps_vt[:, 0:64], t2, vt, start=True, stop=True)  # t2^T V^T
                v = small.tile([64, 64], dt, tag="v")
                nc.scalar.copy(v, ps_v[:, 0:64])
                vt_new = small.tile([64, 64], dt, tag="vt")
                nc.scalar.copy(vt_new, ps_vt[:, 0:64])
                vt = vt_new

            # ZT = 2 * V @ Q^T  [64, F]
            ps_zt = ps_once.tile([128, 512], dt, tag="ns_ps")
            nc.tensor.matmul(ps_zt[0:64, :], vt, qt_ap, start=True, stop=True)
            ztm = const.tile([64, 512], f32r)
            nc.scalar.mul(ztm, ps_zt[0:64, :], 2.0)

            # ---- phase 1: stream x tiles in groups of 4 ----
            # float32r (TF32-like single-pass fp32 matmul) on the correction
            # path; the residual add keeps x in full fp32. Transposes stay in
            # fp32 transpose-mode (their PE stream cost is 2 cyc/row, and the
            # inputs come from DMA which cannot produce f32r-rounded data).
            GT = 4
            x_t = x_d[:, :].rearrange("(t p) f -> t p f", p=128)
            o_t = out_d[:, :].rearrange("(t p) f -> t p f", p=128)
            for g in range(NTILES // GT):
                xi_grp = []
                # xt4 layout [128, (c t n)]: chunk c of all GT tiles adjacent so
                # mm1's rhs for chunk c is the contiguous slice [:, 512c:512c+512]
                xt4 = xts.tile([128, GT * 512], f32r, tag="xt4")
                xt4_v = xt4[:, :].rearrange("p (c t n) -> p c t n", c=4, t=GT)
                for t in range(GT):
                    xi = xs.tile([128, 512], dt, tag="xi")
                    nc.sync.dma_start(xi, x_t[GT * g + t])
                    xi_grp.append(xi)
                    ps_xt = ps_str.tile([128, 512], dt, tag="ps_xt")
                    for c in range(4):
                        nc.tensor.transpose(
                            ps_xt[:, 128 * c : 128 * (c + 1)],
                            xi[:, 128 * c : 128 * (c + 1)],
                            ident,
                        )
                    if t == 3:
                        nc.vector.tensor_copy(
                            xt4_v[:, :, t, :],
                            ps_xt[:, :].rearrange("p (c n) -> p c n", c=4),
                        )
                    else:
                        nc.scalar.copy(
                            xt4_v[:, :, t, :],
                            ps_xt[:, :].rearrange("p (c n) -> p c n", c=4),
                        )
                ps_u4 = ps_u_pool.tile([64, 512], dt, tag="ps_u2")
                for c in range(4):
                    nc.tensor.matmul(
                        ps_u4,
                        wm[:, 64 * c : 64 * (c + 1)],
                        xt4[:, 512 * c : 512 * (c + 1)],
                        start=(c == 0),
                        stop=(c == 3),
                    )
                u4 = us.tile([64, 512], f32r, tag="u2")
                nc.scalar.copy(u4, ps_u4)
                for t in range(GT):
                    ps_o = ps_o_pool.tile([128, 512], dt, tag="ps_o")
                    nc.tensor.matmul(
                        ps_o,
                        u4[:, 128 * t : 128 * (t + 1)],
                        ztm,
                        start=True,
                        stop=True,
                    )
                    ob = outs.tile([128, 512], dt, tag="ob")
                    nc.vector.tensor_add(ob, xi_grp[t], ps_o)
                    nc.sync.dma_start(o_t[GT * g + t], ob)

    return nc


def make_setup(coeff_b, gate_b, coeff_l_b, gate_l_b, comm_b,
               left, right, left_local, right_local):
    """Pack all small inputs for one batch item into one [128, 708] tensor.
    Pure marshalling: transposes/replication of raw inputs plus constants."""
    f32 = np.float32
    s = np.zeros((128, SETUP_COLS), f32)
    s[:, 0:512] = np.concatenate(
        [right.T, left.T, right_local.T, left_local.T,
         left.T, right.T, left_local.T, right_local.T], axis=0
    )
    s[:, _C_IDENT:_C_IDENT + 128] = np.eye(128, dtype=f32)
    s[0:32, _C_E0 + 32:_C_E0 + 64] = -1.0 / 24.0
    s[32:64, _C_E0:_C_E0 + 32] = 1.0 / 24.0
    ones16 = np.ones(16, f32)
    s[:, _C_BASE] = np.concatenate(
        [ones16, coeff_b, ones16, coeff_l_b, coeff_b, ones16, coeff_l_b, ones16]
    )
    s[:, _C_GATE] = np.concatenate(
        [np.ones(64, f32), np.full(32, gate_b, f32), np.full(32, gate_l_b, f32)]
    )
    s[:, _C_SIGN] = np.concatenate(
        [np.ones(80, f32), -np.ones(16, f32), np.ones(16, f32), -np.ones(16, f32)]
    )
    s[0:64, _C_CVEC] = comm_b
    return s


def make_in_maps(x, coeff, gate, coeff_local, gate_local, comm_scale,
                 left, right, left_local, right_local):
    in_maps = []
    for b in range(x.shape[0]):
        in_maps.append({
            "x": np.ascontiguousarray(x[b]).astype(np.float32),
            "setup": make_setup(coeff[b], gate[b], coeff_local[b], gate_local[b],
                                comm_scale[b], left, right, left_local, right_local),
        })
    return in_maps


def kernel(x, coeff, gate, coeff_local, gate_local, comm_scale,
           left, right, left_local, right_local, _trace=False):
    if "nc" not in _CACHE:
        nc = build_bass()
        nc.finalize()  # Bacc.finalize: compile passes + freeze
        _CACHE["nc"] = nc
    nc = _CACHE["nc"]
    in_maps = make_in_maps(x, coeff, gate, coeff_local, gate_local, comm_scale,
                           left, right, left_local, right_local)
    res = run_bass_kernel_spmd(nc, in_maps, core_ids=list(range(8)), trace=_trace)
    out = np.stack([r["out"] for r in res.results], axis=0)
    if _trace:
        _CACHE["last_results"] = res
    return out.astype(x.dtype)


# revision 22
# speedup vs baseline: 1.1739x; 1.0493x over previous
"""Trainium2 Bass kernel for nn_LowRankOrthogonalMixer.

Math: the reference builds per-batch skew matrices G = gate*(A - A^T) with
A = (left*coeff) @ right^T (rank<=16 each term), combines them into
Omega = 0.5*(G+L) + comm/12*(LG-GL), applies the Cayley transform
T = (I-0.5*Omega)^{-1}(I+0.5*Omega), and mixes: out = x @ T.

Key structure: Omega = P @ S @ Q^T with P,Q in [F,64] (rank<=64), so with
W = 0.5*P*S (0.5*Omega = W Q^T) and C = I64 - Q^T W, Woodbury gives
    T = I + 2 W C^{-1} Q^T
exactly, and
    out = x + (x @ W) @ (2 C^{-1}) @ Q^T.
C (64x64) is inverted on-device by Newton-Schulz iteration (C has
eigenvalues 1 +/- 0.5i*mu, well conditioned).

Sharding: data-parallel over batch B=8 -> one batch item per NeuronCore.
"""

import numpy as np

import concourse.bass as bass
import concourse.bacc as bacc
import concourse.tile as tile
from concourse import mybir
from concourse.bass_utils import run_bass_kernel_spmd

B, N, F, R = 8, 4096, 512, 16
NTILES = N // 128
ALPHA = 1.0 / 9.0  # Newton-Schulz init scale: V0 = ALPHA * C^T (safe: sigma_max(C)^2 < 2/ALPHA;
# measured sigma(C) in [0.55, 2.9] across batches -> alpha*sigma_max^2 = 0.93, margin 2.1x)
NS_ITERS = 10

# packed setup tensor column layout
_C_SMALLS = 0        # [128, 512]: rows 0:64 Q^T sources, 64:128 P^T sources
_C_IDENT = 512       # [128, 128] identity
_C_E0 = 640          # [64, 64] +-1/24 commutator mask (rows 64:128 zero)
_C_BASE = 704        # qp_base column
_C_GATE = 705        # qp_gate column
_C_SIGN = 706        # qp_sign column
_C_CVEC = 707        # comm_scale broadcast column (rows 0:64)
SETUP_COLS = 708

_CACHE = {}


def build_bass():
    # Bacc (not plain Bass): its compile() runs move_matmul_waits_to_ldweights
    # + generate_event_semaphores, required because TRN2 instructions support
    # at most one semaphore wait each.
    nc = bacc.Bacc(trn_type="TRN2", target_bir_lowering=False)
    dt = mybir.dt.float32
    bf16 = mybir.dt.bfloat16
    f32r = mybir.dt.float32r

    x_d = nc.dram_tensor("x", [N, F], dt, kind="ExternalInput")
    setup_d = nc.dram_tensor("setup", [128, SETUP_COLS], dt, kind="ExternalInput")
    out_d = nc.dram_tensor("out", [N, F], dt, kind="ExternalOutput")

    with tile.TileContext(nc) as tc:
        with (
            tc.tile_pool(name="const", bufs=1) as const,
            tc.tile_pool(name="small", bufs=2) as small,
            # deep buffering: phase 1's front (DMA/transpose/mm1) must be able
            # to run ~NTILES tiles ahead while phase 0's Newton-Schulz chain
            # (which gates mm2 via ztm) completes
            tc.tile_pool(name="xs", bufs=NTILES + 2) as xs,
            tc.tile_pool(name="xts", bufs=6) as xts,
            tc.tile_pool(name="us", bufs=NTILES // 4 + 2) as us,
            tc.tile_pool(name="outs", bufs=8) as outs,
            tc.tile_pool(name="ps_sm", bufs=2, space="PSUM") as ps_sm,
            tc.tile_pool(name="ps_str", bufs=2, space="PSUM") as ps_str,
            tc.tile_pool(name="ps_u", bufs=2, space="PSUM") as ps_u_pool,
            tc.tile_pool(name="ps_o", bufs=2, space="PSUM") as ps_o_pool,
        ):
            ps_once = ps_sm
            # PE warm-up: ~6us of dense dummy matmuls so the HAM clock gate
            # opens (K=8/8, 2.4 GHz) early; otherwise the whole kernel can run
            # at the cold 1.2 GHz PE clock.
            warm_src = const.tile([128, 128], bf16)
            nc.vector.memset(warm_src, 0.0)
            ps_warm = ps_sm.tile([128, 512], dt, tag="ns_ps")
            for _ in range(56):
                nc.tensor.matmul(ps_warm[:, 0:128], warm_src, warm_src,
                                 start=True, stop=True)

            # ---- load packed setup (1 DMA) + base-partition-0 copy of the P^T half ----
            setup = const.tile([128, SETUP_COLS], dt)
            nc.sync.dma_start(setup, setup_d[:, :])
            setup_p = const.tile([64, SETUP_COLS], dt)
            nc.sync.dma_start(setup_p, setup_d[64:128, :])

            smalls = setup[:, _C_SMALLS:_C_SMALLS + 512]
            ident = setup[:, _C_IDENT:_C_IDENT + 128]
            i64 = setup[0:64, _C_IDENT:_C_IDENT + 64]
            e0 = setup[0:64, _C_E0:_C_E0 + 64]
            base_v = setup[:, _C_BASE:_C_BASE + 1]
            gate_v = setup[:, _C_GATE:_C_GATE + 1]
            sign_v = setup[:, _C_SIGN:_C_SIGN + 1]
            cv = setup[0:64, _C_CVEC:_C_CVEC + 1]

            # ---- phase 0: build W (natural), Q^T, C, C^{-1}, ZT = 2 C^{-1} Q^T ----
            # qp rows 0:64 = Q^T, rows 64:128 = P^T
            scale = small.tile([128, 1], dt, tag="scale")
            nc.vector.tensor_mul(scale, base_v, gate_v)
            scale2 = small.tile([128, 1], dt, tag="scale2")
            nc.vector.tensor_mul(scale2, scale, sign_v)
            qp = const.tile([128, F], dt)
            nc.vector.tensor_scalar_mul(qp, in0=smalls, scalar1=scale2)
            qt_ap = qp[0:64, :]
            # P^T at base partition 0 for the W^T matmul
            scale_p = small.tile([64, 1], dt, tag="scale_p")
            nc.vector.tensor_mul(
                scale_p,
                setup_p[:, _C_BASE:_C_BASE + 1],
                setup_p[:, _C_GATE:_C_GATE + 1],
            )
            scale_p2 = small.tile([64, 1], dt, tag="scale_p2")
            nc.vector.tensor_mul(scale_p2, scale_p, setup_p[:, _C_SIGN:_C_SIGN + 1])
            pt0 = const.tile([64, F], f32r)
            nc.vector.tensor_scalar_mul(
                pt0, in0=setup_p[:, _C_SMALLS:_C_SMALLS + 512], scalar1=scale_p2
            )

            # naturals: qpn block c (cols 128c..128c+128) = (qp[:,128c:128c+128])^T
            ps_qpn = ps_once.tile([128, 512], dt, tag="ns_ps")
            for c in range(4):
                nc.tensor.transpose(
                    ps_qpn[:, 128 * c : 128 * (c + 1)],
                    qp[:, 128 * c : 128 * (c + 1)],
                    ident,
                )
            qpn = const.tile([128, 512], f32r)
            nc.scalar.copy(qpn, ps_qpn)

            # G1 = Q^T P, G1T = P^T Q   (accumulate over 4 F-chunks)
            ps_g1 = ps_sm.tile([64, 128], dt, tag="ns_ps")
            for c in range(4):
                qch = qpn[:, 128 * c : 128 * c + 64]
                pch = qpn[:, 128 * c + 64 : 128 * (c + 1)]
                nc.tensor.matmul(ps_g1[:, 0:64], qch, pch, start=(c == 0), stop=(c == 3))
            g1 = small.tile([64, 64], dt, tag="g1")  # feeds DVE only
            nc.scalar.copy(g1, ps_g1[:, 0:64])
            ps_g1t = ps_sm.tile([64, 128], dt, tag="ns_ps")
            for c in range(4):
                qch = qpn[:, 128 * c : 128 * c + 64]
                pch = qpn[:, 128 * c + 64 : 128 * (c + 1)]
                nc.tensor.matmul(ps_g1t[:, 0:64], pch, qch, start=(c == 0), stop=(c == 3))
            g1t = small.tile([64, 64], f32r, tag="g1t")
            nc.scalar.copy(g1t, ps_g1t[:, 0:64])

            # S_half = 0.25*I + comm * (e0 ⊙ G1)   (e0 carries the ±1/24 pattern)
            e0c = small.tile([64, 64], dt, tag="e0c")
            nc.vector.tensor_scalar_mul(e0c, in0=e0, scalar1=cv)
            s_half = small.tile([64, 64], f32r, tag="s_half")
            nc.vector.tensor_mul(s_half, e0c, g1)
            i4 = small.tile([64, 64], dt, tag="i4")
            nc.scalar.mul(i4, i64, 0.25)
            nc.vector.tensor_add(s_half, s_half, i4)

            # C = I - G1 @ S_half ;  C^T = I - S_half^T @ G1^T
            ps_cr = ps_sm.tile([64, 128], dt, tag="ns_ps")
            nc.tensor.matmul(ps_cr[:, 0:64], g1t, s_half, start=True, stop=True)
            cmat = small.tile([64, 64], f32r, tag="cmat")
            nc.vector.tensor_sub(cmat, i64, ps_cr[:, 0:64])
            ps_ctr = ps_sm.tile([64, 128], dt, tag="ns_ps")
            nc.tensor.matmul(ps_ctr[:, 0:64], s_half, g1t, start=True, stop=True)
            ctm = small.tile([64, 64], f32r, tag="ctm")
            nc.vector.tensor_sub(ctm, i64, ps_ctr[:, 0:64])

            # W^T = S_half^T @ P^T  [64, F]; then W natural in 4 chunks [128, 64]
            ps_wt = ps_once.tile([128, 512], dt, tag="ns_ps")
            nc.tensor.matmul(ps_wt[0:64, :], s_half, pt0, start=True, stop=True)
            wtm = const.tile([64, 512], dt)
            nc.scalar.copy(wtm, ps_wt[0:64, :])
            ps_w = ps_once.tile([128, 512], dt, tag="ns_ps")
            for c in range(4):
                nc.tensor.transpose(
                    ps_w[:, 64 * c : 64 * (c + 1)],
                    wtm[:, 128 * c : 128 * (c + 1)],
                    i64,
                )
            wm = const.tile([128, 256], f32r)
            nc.scalar.copy(wm, ps_w[:, 0:256])

            # Newton-Schulz for V = C^{-1} (maintains V and V^T)
            i2 = small.tile([64, 64], f32r, tag="i2")
            nc.scalar.mul(i2, i64, 2.0)
            v = small.tile([64, 64], f32r, tag="v")
            nc.scalar.mul(v, ctm, ALPHA)
            vt = small.tile([64, 64], f32r, tag="vt")
            nc.scalar.mul(vt, cmat, ALPHA)
            for it in range(NS_ITERS):
                last = it == NS_ITERS - 1
                fp = (lambda ap: ap.bitcast(dt)) if last else (lambda ap: ap)
                ps_t1 = ps_sm.tile([64, 128], dt, tag="ns_ps")
                nc.tensor.matmul(ps_t1[:, 0:64], fp(ctm), fp(v), start=True, stop=True)  # C V
                t2 = small.tile([64, 64], f32r, tag="t2")
                nc.vector.tensor_sub(t2, i2.bitcast(dt) if last else i2, ps_t1[:, 0:64])  # 2I - CV
                ps_v = ps_sm.tile([64, 128], dt, tag="ns_ps")
                nc.tensor.matmul(ps_v[:, 0:64], fp(vt), fp(t2), start=True, stop=True)  # V t2
                ps_vt = ps_sm.tile([64, 128], dt, tag="ns_ps")
                nc.tensor.matmul(ps_vt[:, 0:64], fp(t2), fp(vt), start=True, stop=True)  # t2^T V^T
                v = small.tile([64, 64], f32r, tag="v")
                nc.scalar.copy(v, ps_v[:, 0:64])
                vt_new = small.tile([64, 64], f32r, tag="vt")
                nc.scalar.copy(vt_new, ps_vt[:, 0:64])
                vt = vt_new

            # ZT = 2 * V @ Q^T  [64, F]
            ps_zt = ps_once.tile([128, 512], dt, tag="ns_ps")
            nc.tensor.matmul(ps_zt[0:64, :], vt.bitcast(dt), qt_ap, start=True, stop=True)
            ztm = const.tile([64, 512], f32r)
            nc.scalar.mul(ztm, ps_zt[0:64, :], 2.0)

            # ---- phase 1: stream x tiles in groups of 4 ----
            # float32r (TF32-like single-pass fp32 matmul) on the correction
            # path; the residual add keeps x in full fp32. Transposes stay in
            # fp32 transpose-mode (their PE stream cost is 2 cyc/row, and the
            # inputs come from DMA which cannot produce f32r-rounded data).
            GT = 4
            x_t = x_d[:, :].rearrange("(t p) f -> t p f", p=128)
            o_t = out_d[:, :].rearrange("(t p) f -> t p f", p=128)
            for g in range(NTILES // GT):
                xi_grp = []
                # xt4 layout [128, (c t n)]: chunk c of all GT tiles adjacent so
                # mm1's rhs for chunk c is the contiguous slice [:, 512c:512c+512]
                xt4 = xts.tile([128, GT * 512], f32r, tag="xt4")
                xt4_v = xt4[:, :].rearrange("p (c t n) -> p c t n", c=4, t=GT)
                for t in range(GT):
                    xi = xs.tile([128, 512], dt, tag="xi")
                    nc.sync.dma_start(xi, x_t[GT * g + t])
                    xi_grp.append(xi)
                    ps_xt = ps_str.tile([128, 512], dt, tag="ps_xt")
                    for c in range(4):
                        nc.tensor.transpose(
                            ps_xt[:, 128 * c : 128 * (c + 1)],
                            xi[:, 128 * c : 128 * (c + 1)],
                            ident,
                        )
                    if t >= 2:
                        nc.vector.tensor_copy(
                            xt4_v[:, :, t, :],
                            ps_xt[:, :].rearrange("p (c n) -> p c n", c=4),
                        )
                    else:
                        nc.scalar.copy(
                            xt4_v[:, :, t, :],
                            ps_xt[:, :].rearrange("p (c n) -> p c n", c=4),
                        )
                ps_u4 = ps_u_pool.tile([64, 512], dt, tag="ps_u2")
                for c in range(4):
                    nc.tensor.matmul(
                        ps_u4,
                        wm[:, 64 * c : 64 * (c + 1)],
                        xt4[:, 512 * c : 512 * (c + 1)],
                        start=(c == 0),
                        stop=(c == 3),
                    )
                u4 = us.tile([64, 512], f32r, tag="u2")
                nc.scalar.copy(u4, ps_u4)
                for t in range(GT):
                    ps_o = ps_o_pool.tile([128, 512], dt, tag="ps_o")
                    nc.tensor.matmul(
                        ps_o,
                        u4[:, 128 * t : 128 * (t + 1)],
                        ztm,
                        start=True,
                        stop=True,
                    )
                    ob = outs.tile([128, 512], dt, tag="ob")
                    nc.vector.tensor_add(ob, xi_grp[t], ps_o)
                    nc.sync.dma_start(o_t[GT * g + t], ob)

    return nc


def make_setup(coeff_b, gate_b, coeff_l_b, gate_l_b, comm_b,
               left, right, left_local, right_local):
    """Pack all small inputs for one batch item into one [128, 708] tensor.
    Pure marshalling: transposes/replication of raw inputs plus constants."""
    f32 = np.float32
    s = np.zeros((128, SETUP_COLS), f32)
    s[:, 0:512] = np.concatenate(
        [right.T, left.T, right_local.T, left_local.T,
         left.T, right.T, left_local.T, right_local.T], axis=0
    )
    s[:, _C_IDENT:_C_IDENT + 128] = np.eye(128, dtype=f32)
    s[0:32, _C_E0 + 32:_C_E0 + 64] = -1.0 / 24.0
    s[32:64, _C_E0:_C_E0 + 32] = 1.0 / 24.0
    ones16 = np.ones(16, f32)
    s[:, _C_BASE] = np.concatenate(
        [ones16, coeff_b, ones16, coeff_l_b, coeff_b, ones16, coeff_l_b, ones16]
    )
    s[:, _C_GATE] = np.concatenate(
        [np.ones(64, f32), np.full(32, gate_b, f32), np.full(32, gate_l_b, f32)]
    )
    s[:, _C_SIGN] = np.concatenate(
        [np.ones(80, f32), -np.ones(16, f32), np.ones(16, f32), -np.ones(16, f32)]
    )
    s[0:64, _C_CVEC] = comm_b
    return s


def make_in_maps(x, coeff, gate, coeff_local, gate_local, comm_scale,
                 left, right, left_local, right_local):
    in_maps = []
    for b in range(x.shape[0]):
        in_maps.append({
            "x": np.ascontiguousarray(x[b]).astype(np.float32),
            "setup": make_setup(coeff[b], gate[b], coeff_local[b], gate_local[b],
                                comm_scale[b], left, right, left_local, right_local),
        })
    return in_maps


def kernel(x, coeff, gate, coeff_local, gate_local, comm_scale,
           left, right, left_local, right_local, _trace=False):
    if "nc" not in _CACHE:
        nc = build_bass()
        nc.finalize()  # Bacc.finalize: compile passes + freeze
        _CACHE["nc"] = nc
    nc = _CACHE["nc"]
    in_maps = make_in_maps(x, coeff, gate, coeff_local, gate_local, comm_scale,
                           left, right, left_local, right_local)
    res = run_bass_kernel_spmd(nc, in_maps, core_ids=list(range(8)), trace=_trace)
    out = np.stack([r["out"] for r in res.results], axis=0)
    if _trace:
        _CACHE["last_results"] = res
    return out.astype(x.dtype)
